# revision 33
# baseline (speedup 1.0000x reference)
"""Mamba block Trainium2 kernel, 8-way tensor-parallel over d_inner.

Shapes (hardcoded from the problem spec):
  hidden_states [2, 1024, 1024], d_model=1024, d_inner=2048, d_state=16,
  dt_rank=64, d_conv=4.  Each core owns DL=256 d_inner channels.

v4: chunk-major scan (512-token chunks, h chained via persistent per-state
tiles) so each chunk's out_proj+ReduceScatter overlaps the next chunk's
scan loop and the first scan starts ~50us in.  B/C broadcast via PE
selector matmuls -> PSUM -> Act copies -> SBUF f16 (no DMA broadcast).
All scan-phase elementwise work on DVE (@512 f16 fast path) + Act; the
Pool engine is left idle on purpose: concurrent GpSimd ops halve DVE
throughput (shared SBUF ports).  AR1 per (batch, chunk).
"""
import sys, os
sys.path.insert(0, "/opt/trn_rl_repo")
import numpy as np

import concourse.bass as bass
import concourse.bacc as bacc
import concourse.mybir as mybir
import concourse.tile as tile

F32 = mybir.dt.float32
F16 = mybir.dt.float16
BF16 = mybir.dt.bfloat16
AF = mybir.ActivationFunctionType
OP = mybir.AluOpType

B, L, D, DI, NST, RNK, KC = 2, 1024, 1024, 2048, 16, 64, 4
NC_ = 8
DL = DI // NC_          # 256 local channels
T = B * L               # 2048 tokens
HC = 512                # chunk length (tokens)


def build_nc():
    nc = bacc.Bacc()
    dp = nc.declare_dram_parameter
    hsT = dp("hsT", [D, T], F16, isOutput=False)             # hidden^T fp16
    wxz = dp("wxzT", [8, 128, 512], F16, isOutput=False)     # in_proj^T k-tiles
    xpw = dp("xpwT", [2, 128, 96], F16, isOutput=False)      # x_proj^T k-tiles
    dtw = dp("dtwT", [RNK, DL], F16, isOutput=False)         # dt_proj^T
    wo = dp("woT", [2, 128, D], F16, isOutput=False)         # out_proj^T k-tiles
    cw = dp("convw", [2, 128, KC], F32, isOutput=False)
    cbh = dp("convbh", [2, 128, 1], F32, isOutput=False)     # conv_b / 2
    db2 = dp("dtb2p", [2, 128, 1], F32, isOutput=False)      # +2*dt_proj_b
    av = dp("Aneg", [2, 128, NST], F32, isOutput=False)      # -exp(A_log)
    dgd = dp("diagD", [2, 128, 128], F16, isOutput=False)    # diag(D) per d-tile
    idm = dp("ident", [128, 128], F32, isOutput=False)
    selbc = dp("selbc", [2, 256], F16, isOutput=False)
    outp = dp("out", [B, 128, D], F16, isOutput=True)        # per-core RS slice

    ar1_in = [[nc.dram_tensor(f"ar1_in{b}_{c}", [96, HC], F16)
               for c in range(2)] for b in range(B)]
    ar1_out = [[nc.dram_tensor(f"ar1_out{b}_{c}", [96, HC], F16,
                               addr_space="Shared")
                for c in range(2)] for b in range(B)]
    ar2_in = [nc.dram_tensor(f"ar2_in{b}", [L, D], F16) for b in range(B)]
    ar2_out = [nc.dram_tensor(f"ar2_out{b}", [128, D], F16) for b in range(B)]
    ar_warm_in = nc.dram_tensor("ar_warm_in", [1, 16], F16)
    ar_warm_out = nc.dram_tensor("ar_warm_out", [1, 16], F16, addr_space="Shared")

    PADL = L + KC - 1  # 1027 per batch segment

    with tile.TileContext(nc) as tc:
        with tc.tile_pool(name="wp", bufs=1) as wp, \
             tc.tile_pool(name="data", bufs=1) as dpool, \
             tc.tile_pool(name="stream", bufs=6) as stream, \
             tc.tile_pool(name="scan", bufs=3) as scp, \
             tc.tile_pool(name="bcp", bufs=3) as bcp, \
             tc.tile_pool(name="cvp", bufs=2) as cvp, \
             tc.tile_pool(name="stage", bufs=4) as stg, \
             tc.tile_pool(name="ps", bufs=2, space="PSUM") as ps, \
             tc.tile_pool(name="psb", bufs=2, space="PSUM") as psb, \
             tc.tile_pool(name="psy", bufs=4, space="PSUM") as psy:

            # warm up the collective mesh immediately: the first collective
            # pays ~45us of one-time mesh setup, so fire a tiny one first
            if not os.environ.get("MAMBA_NO_AR"):
                warm = wp.tile([1, 16], F16)
                nc.vector.memset(warm[:], 0.0)
                nc.sync.dma_start(ar_warm_in[:], warm[:])
                nc.gpsimd.collective_compute(
                    "AllReduce", OP.add,
                    replica_groups=[list(range(NC_))],
                    ins=[ar_warm_in[:]], outs=[ar_warm_out[:]])

            # ---- weights / constants ----
            wxz_sb = wp.tile([128, 8 * 512], F16)
            for k in range(8):
                nc.sync.dma_start(wxz_sb[:, 512 * k:512 * (k + 1)], wxz[k])
            xpw_sb = wp.tile([128, 2 * 96], F16)
            dtw_sb = wp.tile([RNK, DL], F16)
            nc.sync.dma_start(dtw_sb[:], dtw[:])
            wo_sb = wp.tile([128, 2 * D], F16)
            cw_sb = wp.tile([128, 2 * KC], F32)
            cbh_sb = wp.tile([128, 2], F32)
            db2_sb = wp.tile([128, 2], F32)
            av_sb = wp.tile([128, 2 * NST], F32)
            dgd_sb = wp.tile([128, 2 * 128], F16)
            for k in range(2):
                nc.sync.dma_start(xpw_sb[:, 96 * k:96 * (k + 1)], xpw[k])
                nc.sync.dma_start(wo_sb[:, D * k:D * (k + 1)], wo[k])
                nc.sync.dma_start(cw_sb[:, KC * k:KC * (k + 1)], cw[k])
                nc.sync.dma_start(cbh_sb[:, k:k + 1], cbh[k])
                nc.sync.dma_start(db2_sb[:, k:k + 1], db2[k])
                nc.sync.dma_start(av_sb[:, NST * k:NST * (k + 1)], av[k])
                nc.sync.dma_start(dgd_sb[:, 128 * k:128 * (k + 1)], dgd[k])
            id_sb = wp.tile([128, 128], F32)
            nc.sync.dma_start(id_sb[:], idm[:])
            idb = wp.tile([128, 128], F16)
            nc.vector.tensor_copy(idb[:], id_sb[:])
            selBC = wp.tile([2, 256], F16)
            nc.sync.dma_start(selBC[:], selbc[:])
            selB = selBC[:, 0:128]
            selC = selBC[:, 128:256]

            # ---- persistent activations ----
            xpad = [dpool.tile([128, B * PADL], F16, name=f"xpad{i}") for i in range(2)]
            zs = [dpool.tile([128, T], F16, name=f"zs{i}") for i in range(2)]
            u16 = [dpool.tile([128, T], F16, name=f"u16_{i}") for i in range(2)]
            delta16 = [dpool.tile([128, T], F16, name=f"delta16_{i}") for i in range(2)]
            du = [dpool.tile([128, T], F16, name=f"du{i}") for i in range(2)]
            yg16 = [dpool.tile([128, T], F16, name=f"yg{i}") for i in range(2)]
            xdbl = [dpool.tile([RNK, L], F16, name=f"xdbl{i}") for i in range(2)]
            hkt = [[dpool.tile([128, 512], F16, name=f"hkt{p}_{k}")
                    for k in range(8)] for p in range(2)]
            # chunk-0 scan states, chained into chunk 1 (per dt, per n)
            hsave = [[dpool.tile([128, HC], F16, name=f"hs{d}_{n}")
                      for n in range(NST)] for d in range(2)]

            cwv = cw_sb.rearrange("p (k m) -> p k m", k=2)
            wxzv = wxz_sb.rearrange("p (k m) -> p k m", k=8)
            wov = wo_sb.rearrange("p (k m) -> p k m", k=2)

            # zero the conv pads
            for dt_ in range(2):
                xp3 = xpad[dt_].rearrange("p (s l) -> p s l", s=B)
                nc.vector.memset(xp3[:, :, 0:KC - 1], 0.0)

            # ---- phase 1: in_proj, one 512-token block, 2 PSUM banks ----
            def in_proj_tb(tb, pair=None):
                if pair is None:
                    in_proj_tb(tb, pair=0)
                    in_proj_tb(tb, pair=1)
                    return
                hk = hkt[tb % 2]
                if pair == 0:
                    for k in range(8):
                        nc.sync.dma_start(
                            hk[k][:],
                            hsT[128 * k:128 * (k + 1), 512 * tb:512 * (tb + 1)])
                # x halves then z halves so only 2 banks are live at a time
                for pair in [pair]:       # 0: x0,x1   1: z0,z1
                    psx = [ps.tile([128, 512], F32, name=f"psx{pair}{i}", tag="ps")
                           for i in range(2)]
                    for k in range(8):
                        for i in range(2):
                            half = 2 * pair + i
                            nc.tensor.matmul(
                                psx[i][:],
                                wxzv[:, k, 128 * half:128 * (half + 1)],
                                hk[k][:],
                                start=(k == 0), stop=(k == 7))
                    for i in range(2):
                        dt_ = i
                        if pair == 0:   # x -> padded conv layout (f16)
                            b_ = tb // 2
                            off = b_ * PADL + (KC - 1) + 512 * (tb % 2)
                            nc.scalar.copy(xpad[dt_][:, off:off + 512], psx[i][:])
                        else:           # z -> silu(z) = (tanh(z/2)+1) * (z/2)
                            sl5 = slice(512 * tb, 512 * (tb + 1))
                            sg = stream.tile([128, 512], F16, name="sg", tag="sg",
                                             bufs=2)
                            nc.scalar.activation(sg[:], psx[i][:], AF.Tanh, scale=0.5)
                            nc.scalar.activation(zs[dt_][:, sl5], psx[i][:],
                                                 AF.Identity, scale=0.5)
                            nc.vector.scalar_tensor_tensor(
                                zs[dt_][:, sl5], sg[:], 1.0, zs[dt_][:, sl5],
                                op0=OP.add, op1=OP.mult)

            # ---- phase 2: depthwise causal conv + silu -> u16, per chunk ----
            def conv_bc(b_, c_):
                tsl = slice(L * b_ + HC * c_, L * b_ + HC * (c_ + 1))
                x0 = b_ * PADL + HC * c_   # xpad col of (token - (KC-1))
                for dt_ in range(2):
                    eng = nc.vector
                    cy0 = cvp.tile([128, HC], F32, name="cy0", tag="cv0")
                    cy1 = cvp.tile([128, HC], F32, name="cy1", tag="cv1")
                    eng.tensor_scalar_mul(cy0[:], xpad[dt_][:, x0:x0 + HC],
                                          cwv[:, dt_, 0:1])
                    abuf = [cy0, cy1]
                    for k in range(1, KC):
                        eng.scalar_tensor_tensor(
                            abuf[k % 2][:], xpad[dt_][:, x0 + k:x0 + k + HC],
                            cwv[:, dt_, k:k + 1],
                            abuf[(k + 1) % 2][:], op0=OP.mult, op1=OP.add)
                    accf = abuf[(KC - 1) % 2]   # cy1
                    sgtf = abuf[KC % 2]         # cy0
                    # u = (acc+cb)*sigmoid(acc+cb) = (tanh(acc/2+cb/2)+1)*(acc/2+cb/2)
                    nc.scalar.activation(sgtf[:], accf[:], AF.Tanh, scale=0.5,
                                         bias=cbh_sb[:, dt_:dt_ + 1])
                    nc.scalar.activation(u16[dt_][:, tsl], accf[:], AF.Identity,
                                         scale=0.5, bias=cbh_sb[:, dt_:dt_ + 1])
                    nc.vector.scalar_tensor_tensor(
                        u16[dt_][:, tsl], sgtf[:], 1.0, u16[dt_][:, tsl],
                        op0=OP.add, op1=OP.mult)

            # ---- phase 3: x_proj partial -> per-(batch,chunk) AllReduce#1 ----
            def xproj_ar1(b_, c_):
                tb = 2 * b_ + c_
                ps96 = ps.tile([96, 512], F32, name="ps96", tag="ps")
                for k in range(2):
                    nc.tensor.matmul(
                        ps96[:], xpw_sb[:, 96 * k:96 * (k + 1)],
                        u16[k][:, 512 * tb:512 * (tb + 1)],
                        start=(k == 0), stop=(k == 1))
                st = stg.tile([96, 512], F16, name="st_xp", tag="xp")
                nc.scalar.copy(st[:], ps96[:])
                nc.sync.dma_start(ar1_in[b_][c_][:], st[:])
                if os.environ.get("MAMBA_NO_AR"):
                    nc.sync.dma_start(ar1_out[b_][c_][:], ar1_in[b_][c_][:])
                else:
                    nc.gpsimd.collective_compute(
                        "AllReduce", OP.add,
                        replica_groups=[list(range(NC_))],
                        ins=[ar1_in[b_][c_][:]], outs=[ar1_out[b_][c_][:]])
                nc.sync.dma_start(xdbl[b_][:, HC * c_:HC * (c_ + 1)],
                                  ar1_out[b_][c_][0:RNK, :])

            # ---- per (batch, chunk): dt_proj + softplus + du ----
            def emit_delta(b_, c_):
                sl5 = slice(L * b_ + HC * c_, L * b_ + HC * (c_ + 1))
                for dt_ in range(2):
                    psd = ps.tile([128, 512], F32, name="psd", tag="ps")
                    nc.tensor.matmul(
                        psd[:], dtw_sb[:, 128 * dt_:128 * (dt_ + 1)],
                        xdbl[b_][:, HC * c_:HC * (c_ + 1)],
                        start=True, stop=True)
                    nc.scalar.activation(delta16[dt_][:, sl5], psd[:],
                                         AF.Exp, bias=db2_sb[:, dt_:dt_ + 1])
                    nc.scalar.activation(delta16[dt_][:, sl5], delta16[dt_][:, sl5],
                                         AF.Ln, bias=1.0)
                    nc.vector.tensor_mul(du[dt_][:, sl5], delta16[dt_][:, sl5],
                                         u16[dt_][:, sl5])

            # ---- scan: chunk-major; h chained c0 -> c1 via hsave ----
            def scan_chunk(b_, c_, inject):
                tsl = slice(L * b_ + HC * c_, L * b_ + HC * (c_ + 1))
                py = [psy.tile([128, 512], F32, name=f"py{b_}{c_}_{i}", tag="psy")
                      for i in range(2)]
                ar1v = ar1_out[b_][c_].rearrange("(g r) t -> g r t", r=NST)
                bcn = {}

                def fetch_bc(n):
                    t = bcp.tile([2, HC], F16, name=f"bcn{n}", tag="bcn", bufs=4)
                    nc.sync.dma_start(t[0:2, :], ar1v[4:6, n, :])
                    bcn[n] = t

                fetch_bc(0)
                fetch_bc(1)
                for n in range(NST):
                    if n in inject:
                        inject[n]()
                    if n + 2 < NST:
                        fetch_bc(n + 2)
                    bbc = bcp.tile([128, HC], F16, name="bbc", tag="bbc")
                    cbc = bcp.tile([128, HC], F16, name="cbc", tag="cbc")
                    pB = psb.tile([128, 512], F32, name="pB", tag="psb")
                    nc.tensor.matmul(pB[:], selB[:], bcn[n][:], start=True,
                                     stop=True)
                    nc.scalar.copy(bbc[:], pB[:])
                    pC = psb.tile([128, 512], F32, name="pC", tag="psb")
                    nc.tensor.matmul(pC[:], selC[:], bcn[n][:], start=True,
                                     stop=True)
                    nc.scalar.copy(cbc[:], pC[:])
                    for dt_ in range(2):
                        dA = scp.tile([128, HC], F32, name="dA", tag=f"dA{dt_}",
                                      bufs=2)
                        nc.scalar.activation(
                            dA[:], delta16[dt_][:, tsl], AF.Exp,
                            scale=av_sb[:, NST * dt_ + n:NST * dt_ + n + 1])
                        dbu = scp.tile([128, HC], F16, name="dbu",
                                       tag=f"dbu{dt_}", bufs=3)
                        nc.vector.tensor_mul(dbu[:], du[dt_][:, tsl], bbc[:])
                        if c_ == 0:
                            h = hsave[dt_][n]
                            nc.vector.tensor_tensor_scan(
                                h[:], dA[:], dbu[:], 0.0,
                                op0=OP.mult, op1=OP.add)
                        else:
                            h = scp.tile([128, HC], F16, name="h",
                                         tag=f"h{dt_}", bufs=2)
                            nc.vector.tensor_tensor_scan(
                                h[:], dA[:], dbu[:],
                                hsave[dt_][n][:, HC - 1:HC],
                                op0=OP.mult, op1=OP.add)
                        ch = scp.tile([128, HC], F16, name="ch",
                                      tag=f"ch{dt_}", bufs=2)
                        nc.vector.tensor_mul(ch[:], h[:], cbc[:])
                        nc.tensor.matmul(py[dt_][:], idb[:], ch[:],
                                         start=(n == 0), stop=False)
                return py

            def backend(b_, c_, py):
                # skip connection (closes psum), gate, out_proj, RS
                for dt_ in range(2):
                    nc.tensor.matmul(
                        py[dt_][:],
                        dgd_sb[:, 128 * dt_:128 * (dt_ + 1)],
                        u16[dt_][:, L * b_ + HC * c_:L * b_ + HC * (c_ + 1)],
                        start=False, stop=True)
                for dt_ in range(2):
                    sl5 = slice(L * b_ + HC * c_, L * b_ + HC * (c_ + 1))
                    nc.vector.tensor_mul(yg16[dt_][:, sl5], py[dt_][:],
                                         zs[dt_][:, sl5])
                # out_proj + ReduceScatter, pipelined per 256-token quarter
                for q in range(2):
                    for tt in range(4 * c_ + 2 * q, 4 * c_ + 2 * q + 2):
                        t0 = L * b_ + 128 * tt
                        for mb in range(2):
                            pso = ps.tile([128, 512], F32, name="pso", tag="ps")
                            for k in range(2):
                                nc.tensor.matmul(
                                    pso[:],
                                    yg16[k][:, t0:t0 + 128],
                                    wov[:, k, 512 * mb:512 * (mb + 1)],
                                    start=(k == 0), stop=(k == 1))
                            st = stg.tile([128, 512], F16, name="st_op",
                                          tag="op", bufs=4)
                            nc.scalar.copy(st[:], pso[:])
                            nc.sync.dma_start(
                                ar2_in[b_][128 * tt:128 * (tt + 1),
                                           512 * mb:512 * (mb + 1)], st[:])
                    qsl = slice(512 * c_ + 256 * q, 512 * c_ + 256 * (q + 1))
                    osl = slice(64 * c_ + 32 * q, 64 * c_ + 32 * (q + 1))
                    if os.environ.get("MAMBA_NO_AR"):
                        nc.sync.dma_start(ar2_out[b_][osl, :],
                                          ar2_in[b_][qsl, :][0:32, :])
                    else:
                        nc.gpsimd.collective_compute(
                            "ReduceScatter", OP.add,
                            replica_groups=[list(range(NC_))],
                            ins=[ar2_in[b_][qsl, :]],
                            outs=[ar2_out[b_][osl, :]])
                    nc.sync.dma_start(outp[b_][osl, :], ar2_out[b_][osl, :])

            # ---- emission ----
            in_proj_tb(0, pair=0)
            conv_bc(0, 0)
            xproj_ar1(0, 0)
            in_proj_tb(0, pair=1)
            in_proj_tb(1, pair=0)
            conv_bc(0, 1)
            in_proj_tb(1, pair=1)
            in_proj_tb(2)
            xproj_ar1(0, 1)
            emit_delta(0, 0)
            in_proj_tb(3)
            py00 = scan_chunk(0, 0, inject={
                2: lambda: conv_bc(1, 0),
                5: lambda: xproj_ar1(1, 0),
                8: lambda: emit_delta(0, 1),
                11: lambda: conv_bc(1, 1),
            })
            py01 = scan_chunk(0, 1, inject={
                1: lambda: backend(0, 0, py00),
                4: lambda: xproj_ar1(1, 1),
                8: lambda: emit_delta(1, 0),
            })
            py10 = scan_chunk(1, 0, inject={
                1: lambda: backend(0, 1, py01),
                6: lambda: emit_delta(1, 1),
            })
            py11 = scan_chunk(1, 1, inject={
                1: lambda: backend(1, 0, py10),
            })
            backend(1, 1, py11)
    nc.finalize()
    return nc


def make_in_maps(inputs):
    hs = np.asarray(inputs["hidden_states"], np.float32)
    ipw = np.asarray(inputs["in_proj_w"], np.float32)
    cw = np.asarray(inputs["conv_w"], np.float32)
    cb = np.asarray(inputs["conv_b"], np.float32)
    xpw = np.asarray(inputs["x_proj_w"], np.float32)
    dtw = np.asarray(inputs["dt_proj_w"], np.float32)
    dtb = np.asarray(inputs["dt_proj_b"], np.float32)
    alog = np.asarray(inputs["A_log"], np.float32)
    dvec = np.asarray(inputs["D"], np.float32)
    wo = np.asarray(inputs["out_proj_w"], np.float32)

    hsT = np.ascontiguousarray(hs.transpose(2, 0, 1).reshape(D, T)).astype(np.float16)
    ident = np.eye(128, dtype=np.float32)
    selbc_np = np.zeros((2, 256), np.float16)
    selbc_np[0, 0:128] = 1.0   # selB row 0
    selbc_np[1, 128:256] = 1.0  # selC row 1

    in_maps = []
    for c in range(NC_):
        sl = slice(DL * c, DL * (c + 1))
        wxzT = np.concatenate([ipw[sl].T, ipw[DI + DL * c: DI + DL * (c + 1)].T],
                              axis=1)                      # [1024, 512]
        dloc = dvec[sl]
        dgd = np.stack([np.diag(dloc[0:128]), np.diag(dloc[128:256])])
        m = {
            "hsT": hsT,
            "wxzT": np.ascontiguousarray(wxzT.reshape(8, 128, 512)).astype(np.float16),
            "xpwT": np.ascontiguousarray(xpw[:, sl].T.reshape(2, 128, 96)).astype(np.float16),
            "dtwT": np.ascontiguousarray(dtw[sl].T).astype(np.float16),
            "woT": np.ascontiguousarray(wo[:, sl].T.reshape(2, 128, D)).astype(np.float16),
            "convw": np.ascontiguousarray(cw[sl, 0, :].reshape(2, 128, KC)),
            "convbh": np.ascontiguousarray((0.5 * cb[sl]).reshape(2, 128, 1)),
            "dtb2p": np.ascontiguousarray((2.0 * dtb[sl]).reshape(2, 128, 1)),
            "Aneg": np.ascontiguousarray((-np.exp(alog[sl])).reshape(2, 128, NST)),
            "diagD": np.ascontiguousarray(dgd).astype(np.float16),
            "ident": ident,
            "selbc": selbc_np,
        }
        in_maps.append(m)
    return in_maps


def assemble_output(results):
    # RS per 256-token quarter q of chunk c: core k receives tokens
    # [512c + 256q + 32k, +32), stored at rows [64c + 32q, +32).
    out = np.zeros((B, L, D), np.float32)
    for k in range(NC_):
        s = np.asarray(results[k]["out"], np.float32)  # [B, 128, D]
        for b_ in range(B):
            for c in range(2):
                for q in range(2):
                    t0 = 512 * c + 256 * q + 32 * k
                    out[b_, t0:t0 + 32, :] = s[b_][64 * c + 32 * q:
                                                   64 * c + 32 * q + 32]
    return out


def kernel(**inputs):
    from concourse.bass_utils import run_bass_kernel_spmd
    nc = build_nc()
    in_maps = make_in_maps(inputs)
    trace = bool(int(os.environ.get("MAMBA_TRACE", "0")))
    tdir = os.environ.get("MAMBA_TRACE_DIR") if trace else None
    if tdir:
        os.makedirs(tdir, exist_ok=True)
    res = run_bass_kernel_spmd(nc, in_maps, list(range(NC_)), trace=trace,
                               tmpdir=tdir)
    if trace and res.exec_time_ns is not None:
        print(f"HW exec time: {res.exec_time_ns} ns")
        if res.instructions_and_trace is not None:
            print(f"trace path: {res.instructions_and_trace[1]}")
    return assemble_output(res.results)


# revision 35
# speedup vs baseline: 1.1770x; 1.1770x over previous
"""Mamba block Trainium2 kernel, 8-way tensor-parallel over d_inner.

Shapes (hardcoded from the problem spec):
  hidden_states [2, 1024, 1024], d_model=1024, d_inner=2048, d_state=16,
  dt_rank=64, d_conv=4.  Each core owns DL=256 d_inner channels.

v4: chunk-major scan (512-token chunks, h chained via persistent per-state
tiles) so each chunk's out_proj+ReduceScatter overlaps the next chunk's
scan loop and the first scan starts ~50us in.  B/C broadcast via PE
selector matmuls -> PSUM -> Act copies -> SBUF f16 (no DMA broadcast).
All scan-phase elementwise work on DVE (@512 f16 fast path) + Act; the
Pool engine is left idle on purpose: concurrent GpSimd ops halve DVE
throughput (shared SBUF ports).  AR1 per (batch, chunk).
"""
import sys, os
sys.path.insert(0, "/opt/trn_rl_repo")
import numpy as np

import concourse.bass as bass
import concourse.bacc as bacc
import concourse.mybir as mybir
import concourse.tile as tile

F32 = mybir.dt.float32
F16 = mybir.dt.float16
BF16 = mybir.dt.bfloat16
AF = mybir.ActivationFunctionType
OP = mybir.AluOpType

B, L, D, DI, NST, RNK, KC = 2, 1024, 1024, 2048, 16, 64, 4
NC_ = 8
DL = DI // NC_          # 256 local channels
T = B * L               # 2048 tokens
HC = 512                # chunk length (tokens)


def build_nc():
    nc = bacc.Bacc()
    dp = nc.declare_dram_parameter
    hsT = dp("hsT", [D, T], F16, isOutput=False)             # hidden^T fp16
    wxz = dp("wxzT", [8, 128, 512], F16, isOutput=False)     # in_proj^T k-tiles
    xpw = dp("xpwT", [2, 128, 96], F16, isOutput=False)      # x_proj^T k-tiles
    dtw = dp("dtwT", [RNK, DL], F16, isOutput=False)         # dt_proj^T
    wo = dp("woT", [2, 128, D], F16, isOutput=False)         # out_proj^T k-tiles
    cw = dp("convw", [2, 128, KC], F32, isOutput=False)
    cbh = dp("convbh", [2, 128, 1], F32, isOutput=False)     # conv_b / 2
    db2 = dp("dtb2p", [2, 128, 1], F32, isOutput=False)      # +2*dt_proj_b
    av = dp("Aneg", [2, 128, NST], F32, isOutput=False)      # -exp(A_log)
    dgd = dp("diagD", [2, 128, 128], F16, isOutput=False)    # diag(D) per d-tile
    idm = dp("ident", [128, 128], F32, isOutput=False)
    selbc = dp("selbc", [2, 256], F16, isOutput=False)
    outp = dp("out", [B, 128, D], F16, isOutput=True)        # per-core RS slice

    ar1_in = [[nc.dram_tensor(f"ar1_in{b}_{c}", [96, HC], F16)
               for c in range(2)] for b in range(B)]
    ar1_out = [[nc.dram_tensor(f"ar1_out{b}_{c}", [96, HC], F16,
                               addr_space="Shared")
                for c in range(2)] for b in range(B)]
    ar2_in = [nc.dram_tensor(f"ar2_in{b}", [L, D], F16) for b in range(B)]
    ar2_out = [nc.dram_tensor(f"ar2_out{b}", [128, D], F16) for b in range(B)]
    ar_warm_in = nc.dram_tensor("ar_warm_in", [1, 16], F16)
    ar_warm_out = nc.dram_tensor("ar_warm_out", [1, 16], F16, addr_space="Shared")

    PADL = L + KC - 1  # 1027 per batch segment

    with tile.TileContext(nc) as tc:
        with tc.tile_pool(name="wp", bufs=1) as wp, \
             tc.tile_pool(name="data", bufs=1) as dpool, \
             tc.tile_pool(name="stream", bufs=6) as stream, \
             tc.tile_pool(name="scan", bufs=3) as scp, \
             tc.tile_pool(name="bcp", bufs=3) as bcp, \
             tc.tile_pool(name="cvp", bufs=2) as cvp, \
             tc.tile_pool(name="stage", bufs=4) as stg, \
             tc.tile_pool(name="ps", bufs=2, space="PSUM") as ps, \
             tc.tile_pool(name="psb", bufs=2, space="PSUM") as psb, \
             tc.tile_pool(name="psy", bufs=4, space="PSUM") as psy:

            # warm up the collective mesh immediately: the first collective
            # pays ~45us of one-time mesh setup, so fire a tiny one first
            if not os.environ.get("MAMBA_NO_AR"):
                warm = wp.tile([1, 16], F16)
                nc.vector.memset(warm[:], 0.0)
                nc.sync.dma_start(ar_warm_in[:], warm[:])
                nc.gpsimd.collective_compute(
                    "AllReduce", OP.add,
                    replica_groups=[list(range(NC_))],
                    ins=[ar_warm_in[:]], outs=[ar_warm_out[:]])

            # ---- weights / constants ----
            wxz_sb = wp.tile([128, 8 * 512], F16)
            for k in range(8):
                nc.sync.dma_start(wxz_sb[:, 512 * k:512 * (k + 1)], wxz[k])
            xpw_sb = wp.tile([128, 2 * 96], F16)
            dtw_sb = wp.tile([RNK, DL], F16)
            nc.sync.dma_start(dtw_sb[:], dtw[:])
            wo_sb = wp.tile([128, 2 * D], F16)
            cw_sb = wp.tile([128, 2 * KC], F32)
            cbh_sb = wp.tile([128, 2], F32)
            db2_sb = wp.tile([128, 2], F32)
            av_sb = wp.tile([128, 2 * NST], F32)
            dgd_sb = wp.tile([128, 2 * 128], F16)
            for k in range(2):
                nc.sync.dma_start(xpw_sb[:, 96 * k:96 * (k + 1)], xpw[k])
                nc.sync.dma_start(wo_sb[:, D * k:D * (k + 1)], wo[k])
                nc.sync.dma_start(cw_sb[:, KC * k:KC * (k + 1)], cw[k])
                nc.sync.dma_start(cbh_sb[:, k:k + 1], cbh[k])
                nc.sync.dma_start(db2_sb[:, k:k + 1], db2[k])
                nc.sync.dma_start(av_sb[:, NST * k:NST * (k + 1)], av[k])
                nc.sync.dma_start(dgd_sb[:, 128 * k:128 * (k + 1)], dgd[k])
            id_sb = wp.tile([128, 128], F32)
            nc.sync.dma_start(id_sb[:], idm[:])
            idb = wp.tile([128, 128], F16)
            nc.vector.tensor_copy(idb[:], id_sb[:])
            selBC = wp.tile([2, 256], F16)
            nc.sync.dma_start(selBC[:], selbc[:])
            selB = selBC[:, 0:128]
            selC = selBC[:, 128:256]

            # ---- persistent activations ----
            xpad = [dpool.tile([128, B * PADL], F16, name=f"xpad{i}") for i in range(2)]
            zs = [dpool.tile([128, T], F16, name=f"zs{i}") for i in range(2)]
            u16 = [dpool.tile([128, T], F16, name=f"u16_{i}") for i in range(2)]
            delta16 = [dpool.tile([128, T], F16, name=f"delta16_{i}") for i in range(2)]
            du = [dpool.tile([128, T], F16, name=f"du{i}") for i in range(2)]
            yg16 = [dpool.tile([128, T], F16, name=f"yg{i}") for i in range(2)]
            xdbl = [dpool.tile([RNK, L], F16, name=f"xdbl{i}") for i in range(2)]
            hkt = [[dpool.tile([128, 512], F16, name=f"hkt{p}_{k}")
                    for k in range(8)] for p in range(2)]
            # chunk-0 scan states, chained into chunk 1 (per dt, per n)
            hsave = [[dpool.tile([128, HC], F16, name=f"hs{d}_{n}")
                      for n in range(NST)] for d in range(2)]

            cwv = cw_sb.rearrange("p (k m) -> p k m", k=2)
            wxzv = wxz_sb.rearrange("p (k m) -> p k m", k=8)
            wov = wo_sb.rearrange("p (k m) -> p k m", k=2)

            # zero the conv pads
            for dt_ in range(2):
                xp3 = xpad[dt_].rearrange("p (s l) -> p s l", s=B)
                nc.vector.memset(xp3[:, :, 0:KC - 1], 0.0)

            # ---- phase 1: in_proj, one 512-token block, 2 PSUM banks ----
            def in_proj_tb(tb, pair=None):
                if pair is None:
                    in_proj_tb(tb, pair=0)
                    in_proj_tb(tb, pair=1)
                    return
                hk = hkt[tb % 2]
                if pair == 0:
                    for k in range(8):
                        nc.sync.dma_start(
                            hk[k][:],
                            hsT[128 * k:128 * (k + 1), 512 * tb:512 * (tb + 1)])
                # x halves then z halves so only 2 banks are live at a time
                for pair in [pair]:       # 0: x0,x1   1: z0,z1
                    psx = [ps.tile([128, 512], F32, name=f"psx{pair}{i}", tag="ps")
                           for i in range(2)]
                    for k in range(8):
                        for i in range(2):
                            half = 2 * pair + i
                            nc.tensor.matmul(
                                psx[i][:],
                                wxzv[:, k, 128 * half:128 * (half + 1)],
                                hk[k][:],
                                start=(k == 0), stop=(k == 7))
                    for i in range(2):
                        dt_ = i
                        if pair == 0:   # x -> padded conv layout (f16)
                            b_ = tb // 2
                            off = b_ * PADL + (KC - 1) + 512 * (tb % 2)
                            nc.scalar.copy(xpad[dt_][:, off:off + 512], psx[i][:])
                        else:           # z -> silu(z) = (tanh(z/2)+1) * (z/2)
                            sl5 = slice(512 * tb, 512 * (tb + 1))
                            sg = stream.tile([128, 512], F16, name="sg", tag="sg",
                                             bufs=2)
                            nc.scalar.activation(sg[:], psx[i][:], AF.Tanh, scale=0.5)
                            nc.scalar.activation(zs[dt_][:, sl5], psx[i][:],
                                                 AF.Identity, scale=0.5)
                            nc.vector.scalar_tensor_tensor(
                                zs[dt_][:, sl5], sg[:], 1.0, zs[dt_][:, sl5],
                                op0=OP.add, op1=OP.mult)

            # ---- phase 2: depthwise causal conv + silu -> u16, per chunk ----
            def conv_bc(b_, c_):
                tsl = slice(L * b_ + HC * c_, L * b_ + HC * (c_ + 1))
                x0 = b_ * PADL + HC * c_   # xpad col of (token - (KC-1))
                for dt_ in range(2):
                    eng = nc.vector
                    cy0 = cvp.tile([128, HC], F32, name="cy0", tag="cv0")
                    cy1 = cvp.tile([128, HC], F32, name="cy1", tag="cv1")
                    eng.tensor_scalar_mul(cy0[:], xpad[dt_][:, x0:x0 + HC],
                                          cwv[:, dt_, 0:1])
                    abuf = [cy0, cy1]
                    for k in range(1, KC):
                        eng.scalar_tensor_tensor(
                            abuf[k % 2][:], xpad[dt_][:, x0 + k:x0 + k + HC],
                            cwv[:, dt_, k:k + 1],
                            abuf[(k + 1) % 2][:], op0=OP.mult, op1=OP.add)
                    accf = abuf[(KC - 1) % 2]   # cy1
                    sgtf = abuf[KC % 2]         # cy0
                    # u = (acc+cb)*sigmoid(acc+cb) = (tanh(acc/2+cb/2)+1)*(acc/2+cb/2)
                    nc.scalar.activation(sgtf[:], accf[:], AF.Tanh, scale=0.5,
                                         bias=cbh_sb[:, dt_:dt_ + 1])
                    nc.scalar.activation(u16[dt_][:, tsl], accf[:], AF.Identity,
                                         scale=0.5, bias=cbh_sb[:, dt_:dt_ + 1])
                    nc.vector.scalar_tensor_tensor(
                        u16[dt_][:, tsl], sgtf[:], 1.0, u16[dt_][:, tsl],
                        op0=OP.add, op1=OP.mult)

            # ---- phase 3: x_proj partial -> per-(batch,chunk) AllReduce#1 ----
            def xproj_ar1(b_, c_):
                tb = 2 * b_ + c_
                ps96 = ps.tile([96, 512], F32, name="ps96", tag="ps")
                for k in range(2):
                    nc.tensor.matmul(
                        ps96[:], xpw_sb[:, 96 * k:96 * (k + 1)],
                        u16[k][:, 512 * tb:512 * (tb + 1)],
                        start=(k == 0), stop=(k == 1))
                st = stg.tile([96, 512], F16, name="st_xp", tag="xp")
                nc.scalar.copy(st[:], ps96[:])
                nc.sync.dma_start(ar1_in[b_][c_][:], st[:])
                if os.environ.get("MAMBA_NO_AR"):
                    nc.sync.dma_start(ar1_out[b_][c_][:], ar1_in[b_][c_][:])
                else:
                    nc.gpsimd.collective_compute(
                        "AllReduce", OP.add,
                        replica_groups=[list(range(NC_))],
                        ins=[ar1_in[b_][c_][:]], outs=[ar1_out[b_][c_][:]])
                nc.sync.dma_start(xdbl[b_][:, HC * c_:HC * (c_ + 1)],
                                  ar1_out[b_][c_][0:RNK, :])

            # ---- per (batch, chunk): dt_proj + softplus + du ----
            def emit_delta(b_, c_):
                sl5 = slice(L * b_ + HC * c_, L * b_ + HC * (c_ + 1))
                for dt_ in range(2):
                    psd = ps.tile([128, 512], F32, name="psd", tag="ps")
                    nc.tensor.matmul(
                        psd[:], dtw_sb[:, 128 * dt_:128 * (dt_ + 1)],
                        xdbl[b_][:, HC * c_:HC * (c_ + 1)],
                        start=True, stop=True)
                    nc.scalar.activation(delta16[dt_][:, sl5], psd[:],
                                         AF.Exp, bias=db2_sb[:, dt_:dt_ + 1])
                    nc.scalar.activation(delta16[dt_][:, sl5], delta16[dt_][:, sl5],
                                         AF.Ln, bias=1.0)
                    nc.vector.tensor_mul(du[dt_][:, sl5], delta16[dt_][:, sl5],
                                         u16[dt_][:, sl5])

            # ---- scan: chunk-major; h chained c0 -> c1 via hsave ----
            def scan_chunk(b_, c_, inject):
                tsl = slice(L * b_ + HC * c_, L * b_ + HC * (c_ + 1))
                py = [psy.tile([128, 512], F32, name=f"py{b_}{c_}_{i}", tag="psy")
                      for i in range(2)]
                ar1v = ar1_out[b_][c_].rearrange("(g r) t -> g r t", r=NST)
                bcn = {}

                def fetch_bc(n):
                    t = bcp.tile([2, HC], F16, name=f"bcn{n}", tag="bcn", bufs=4)
                    nc.sync.dma_start(t[0:2, :], ar1v[4:6, n, :])
                    bcn[n] = t

                fetch_bc(0)
                fetch_bc(1)
                for n in range(NST):
                    if n in inject:
                        inject[n]()
                    if n + 2 < NST:
                        fetch_bc(n + 2)
                    bbc = bcp.tile([128, HC], F16, name="bbc", tag="bbc")
                    cbc = bcp.tile([128, HC], F16, name="cbc", tag="cbc")
                    pB = psb.tile([128, 512], F32, name="pB", tag="psb")
                    nc.tensor.matmul(pB[:], selB[:], bcn[n][:], start=True,
                                     stop=True)
                    nc.scalar.copy(bbc[:], pB[:])
                    pC = psb.tile([128, 512], F32, name="pC", tag="psb")
                    nc.tensor.matmul(pC[:], selC[:], bcn[n][:], start=True,
                                     stop=True)
                    nc.scalar.copy(cbc[:], pC[:])
                    for dt_ in range(2):
                        dA = scp.tile([128, HC], F32, name="dA", tag=f"dA{dt_}",
                                      bufs=2)
                        nc.scalar.activation(
                            dA[:], delta16[dt_][:, tsl], AF.Exp,
                            scale=av_sb[:, NST * dt_ + n:NST * dt_ + n + 1])
                        dbu = scp.tile([128, HC], F16, name="dbu",
                                       tag=f"dbu{dt_}", bufs=3)
                        nc.vector.tensor_mul(dbu[:], du[dt_][:, tsl], bbc[:])
                        if c_ == 0:
                            h = hsave[dt_][n]
                            nc.vector.tensor_tensor_scan(
                                h[:], dA[:], dbu[:], 0.0,
                                op0=OP.mult, op1=OP.add)
                        else:
                            h = scp.tile([128, HC], F16, name="h",
                                         tag=f"h{dt_}", bufs=2)
                            nc.vector.tensor_tensor_scan(
                                h[:], dA[:], dbu[:],
                                hsave[dt_][n][:, HC - 1:HC],
                                op0=OP.mult, op1=OP.add)
                        ch = scp.tile([128, HC], F16, name="ch",
                                      tag=f"ch{dt_}", bufs=2)
                        nc.vector.tensor_mul(ch[:], h[:], cbc[:])
                        nc.tensor.matmul(py[dt_][:], idb[:], ch[:],
                                         start=(n == 0), stop=False)
                return py

            def backend(b_, c_, py):
                # skip connection (closes psum), gate, out_proj, RS
                for dt_ in range(2):
                    nc.tensor.matmul(
                        py[dt_][:],
                        dgd_sb[:, 128 * dt_:128 * (dt_ + 1)],
                        u16[dt_][:, L * b_ + HC * c_:L * b_ + HC * (c_ + 1)],
                        start=False, stop=True)
                for dt_ in range(2):
                    sl5 = slice(L * b_ + HC * c_, L * b_ + HC * (c_ + 1))
                    nc.vector.tensor_mul(yg16[dt_][:, sl5], py[dt_][:],
                                         zs[dt_][:, sl5])
                # out_proj matmuls/stores, then one ReduceScatter per chunk
                for tt in range(4 * c_, 4 * (c_ + 1)):
                    t0 = L * b_ + 128 * tt
                    for mb in range(2):
                        pso = ps.tile([128, 512], F32, name="pso", tag="ps")
                        for k in range(2):
                            nc.tensor.matmul(
                                pso[:],
                                yg16[k][:, t0:t0 + 128],
                                wov[:, k, 512 * mb:512 * (mb + 1)],
                                start=(k == 0), stop=(k == 1))
                        st = stg.tile([128, 512], F16, name="st_op",
                                      tag="op", bufs=4)
                        nc.scalar.copy(st[:], pso[:])
                        nc.sync.dma_start(
                            ar2_in[b_][128 * tt:128 * (tt + 1),
                                       512 * mb:512 * (mb + 1)], st[:])
                hsl = slice(512 * c_, 512 * (c_ + 1))
                osl = slice(64 * c_, 64 * (c_ + 1))
                if os.environ.get("MAMBA_NO_AR"):
                    nc.sync.dma_start(ar2_out[b_][osl, :], ar2_in[b_][osl, :])
                else:
                    nc.gpsimd.collective_compute(
                        "ReduceScatter", OP.add,
                        replica_groups=[list(range(NC_))],
                        ins=[ar2_in[b_][hsl, :]], outs=[ar2_out[b_][osl, :]])
                nc.sync.dma_start(outp[b_][osl, :], ar2_out[b_][osl, :])

            # ---- emission ----
            in_proj_tb(0, pair=0)
            conv_bc(0, 0)
            xproj_ar1(0, 0)
            in_proj_tb(0, pair=1)
            in_proj_tb(1, pair=0)
            conv_bc(0, 1)
            in_proj_tb(1, pair=1)
            in_proj_tb(2)
            xproj_ar1(0, 1)
            emit_delta(0, 0)
            in_proj_tb(3)
            py00 = scan_chunk(0, 0, inject={
                2: lambda: conv_bc(1, 0),
                5: lambda: xproj_ar1(1, 0),
                8: lambda: emit_delta(0, 1),
                11: lambda: conv_bc(1, 1),
            })
            py01 = scan_chunk(0, 1, inject={
                1: lambda: backend(0, 0, py00),
                4: lambda: xproj_ar1(1, 1),
                8: lambda: emit_delta(1, 0),
            })
            py10 = scan_chunk(1, 0, inject={
                1: lambda: backend(0, 1, py01),
                6: lambda: emit_delta(1, 1),
            })
            py11 = scan_chunk(1, 1, inject={
                1: lambda: backend(1, 0, py10),
            })
            backend(1, 1, py11)
    nc.finalize()
    return nc


def make_in_maps(inputs):
    hs = np.asarray(inputs["hidden_states"], np.float32)
    ipw = np.asarray(inputs["in_proj_w"], np.float32)
    cw = np.asarray(inputs["conv_w"], np.float32)
    cb = np.asarray(inputs["conv_b"], np.float32)
    xpw = np.asarray(inputs["x_proj_w"], np.float32)
    dtw = np.asarray(inputs["dt_proj_w"], np.float32)
    dtb = np.asarray(inputs["dt_proj_b"], np.float32)
    alog = np.asarray(inputs["A_log"], np.float32)
    dvec = np.asarray(inputs["D"], np.float32)
    wo = np.asarray(inputs["out_proj_w"], np.float32)

    hsT = np.ascontiguousarray(hs.transpose(2, 0, 1).reshape(D, T)).astype(np.float16)
    ident = np.eye(128, dtype=np.float32)
    selbc_np = np.zeros((2, 256), np.float16)
    selbc_np[0, 0:128] = 1.0   # selB row 0
    selbc_np[1, 128:256] = 1.0  # selC row 1

    in_maps = []
    for c in range(NC_):
        sl = slice(DL * c, DL * (c + 1))
        wxzT = np.concatenate([ipw[sl].T, ipw[DI + DL * c: DI + DL * (c + 1)].T],
                              axis=1)                      # [1024, 512]
        dloc = dvec[sl]
        dgd = np.stack([np.diag(dloc[0:128]), np.diag(dloc[128:256])])
        m = {
            "hsT": hsT,
            "wxzT": np.ascontiguousarray(wxzT.reshape(8, 128, 512)).astype(np.float16),
            "xpwT": np.ascontiguousarray(xpw[:, sl].T.reshape(2, 128, 96)).astype(np.float16),
            "dtwT": np.ascontiguousarray(dtw[sl].T).astype(np.float16),
            "woT": np.ascontiguousarray(wo[:, sl].T.reshape(2, 128, D)).astype(np.float16),
            "convw": np.ascontiguousarray(cw[sl, 0, :].reshape(2, 128, KC)),
            "convbh": np.ascontiguousarray((0.5 * cb[sl]).reshape(2, 128, 1)),
            "dtb2p": np.ascontiguousarray((2.0 * dtb[sl]).reshape(2, 128, 1)),
            "Aneg": np.ascontiguousarray((-np.exp(alog[sl])).reshape(2, 128, NST)),
            "diagD": np.ascontiguousarray(dgd).astype(np.float16),
            "ident": ident,
            "selbc": selbc_np,
        }
        in_maps.append(m)
    return in_maps


def assemble_output(results):
    # RS per 512-token chunk c: core k receives tokens [512c + 64k, +64),
    # stored at rows [64c, +64).
    out = np.zeros((B, L, D), np.float32)
    for k in range(NC_):
        s = np.asarray(results[k]["out"], np.float32)  # [B, 128, D]
        for b_ in range(B):
            for c in range(2):
                out[b_, 512 * c + 64 * k: 512 * c + 64 * (k + 1), :] = \
                    s[b_][64 * c:64 * (c + 1)]
    return out


def kernel(**inputs):
    from concourse.bass_utils import run_bass_kernel_spmd
    nc = build_nc()
    in_maps = make_in_maps(inputs)
    trace = bool(int(os.environ.get("MAMBA_TRACE", "0")))
    tdir = os.environ.get("MAMBA_TRACE_DIR") if trace else None
    if tdir:
        os.makedirs(tdir, exist_ok=True)
    res = run_bass_kernel_spmd(nc, in_maps, list(range(NC_)), trace=trace,
                               tmpdir=tdir)
    if trace and res.exec_time_ns is not None:
        print(f"HW exec time: {res.exec_time_ns} ns")
        if res.instructions_and_trace is not None:
            print(f"trace path: {res.instructions_and_trace[1]}")
    return assemble_output(res.results)


# revision 39
# speedup vs baseline: 1.1876x; 1.0090x over previous
"""Mamba block Trainium2 kernel, 8-way tensor-parallel over d_inner.

Shapes (hardcoded from the problem spec):
  hidden_states [2, 1024, 1024], d_model=1024, d_inner=2048, d_state=16,
  dt_rank=64, d_conv=4.  Each core owns DL=256 d_inner channels.

v4: chunk-major scan (512-token chunks, h chained via persistent per-state
tiles) so each chunk's out_proj+ReduceScatter overlaps the next chunk's
scan loop and the first scan starts ~50us in.  B/C broadcast via PE
selector matmuls -> PSUM -> Act copies -> SBUF f16 (no DMA broadcast).
All scan-phase elementwise work on DVE (@512 f16 fast path) + Act; the
Pool engine is left idle on purpose: concurrent GpSimd ops halve DVE
throughput (shared SBUF ports).  AR1 per (batch, chunk).
"""
import sys, os
sys.path.insert(0, "/opt/trn_rl_repo")
import numpy as np

import concourse.bass as bass
import concourse.bacc as bacc
import concourse.mybir as mybir
import concourse.tile as tile

F32 = mybir.dt.float32
F16 = mybir.dt.float16
BF16 = mybir.dt.bfloat16
AF = mybir.ActivationFunctionType
OP = mybir.AluOpType

B, L, D, DI, NST, RNK, KC = 2, 1024, 1024, 2048, 16, 64, 4
NC_ = 8
DL = DI // NC_          # 256 local channels
T = B * L               # 2048 tokens
HC = 512                # chunk length (tokens)


def build_nc():
    nc = bacc.Bacc()
    dp = nc.declare_dram_parameter
    hsT = dp("hsT", [D, T], F16, isOutput=False)             # hidden^T fp16
    wxz = dp("wxzT", [8, 128, 512], F16, isOutput=False)     # in_proj^T k-tiles
    xpw = dp("xpwT", [2, 128, 96], F16, isOutput=False)      # x_proj^T k-tiles
    dtw = dp("dtwT", [RNK, DL], F16, isOutput=False)         # dt_proj^T
    wo = dp("woT", [2, 128, D], F16, isOutput=False)         # out_proj^T k-tiles
    cw = dp("convw", [2, 128, KC], F32, isOutput=False)
    cdg = dp("convdg", [2 * KC, 128, 128], F16, isOutput=False)  # diag taps
    cbh = dp("convbh", [2, 128, 1], F32, isOutput=False)     # conv_b / 2
    db2 = dp("dtb2p", [2, 128, 1], F32, isOutput=False)      # +2*dt_proj_b
    av = dp("Aneg", [2, 128, NST], F32, isOutput=False)      # -exp(A_log)
    dgd = dp("diagD", [2, 128, 128], F16, isOutput=False)    # diag(D) per d-tile
    idm = dp("ident", [128, 128], F32, isOutput=False)
    selbc = dp("selbc", [2, 256], F16, isOutput=False)
    outp = dp("out", [B, 128, D], F16, isOutput=True)        # per-core RS slice

    ar1_in = [[nc.dram_tensor(f"ar1_in{b}_{c}", [96, HC], F16)
               for c in range(2)] for b in range(B)]
    ar1_out = [[nc.dram_tensor(f"ar1_out{b}_{c}", [96, HC], F16,
                               addr_space="Shared")
                for c in range(2)] for b in range(B)]
    ar2_in = [nc.dram_tensor(f"ar2_in{b}", [L, D], F16) for b in range(B)]
    ar2_out = [nc.dram_tensor(f"ar2_out{b}", [128, D], F16) for b in range(B)]
    ar_warm_in = nc.dram_tensor("ar_warm_in", [1, 16], F16)
    ar_warm_out = nc.dram_tensor("ar_warm_out", [1, 16], F16, addr_space="Shared")

    PADL = L + KC - 1  # 1027 per batch segment

    with tile.TileContext(nc) as tc:
        with tc.tile_pool(name="wp", bufs=1) as wp, \
             tc.tile_pool(name="data", bufs=1) as dpool, \
             tc.tile_pool(name="stream", bufs=6) as stream, \
             tc.tile_pool(name="scan", bufs=3) as scp, \
             tc.tile_pool(name="bcp", bufs=3) as bcp, \
             tc.tile_pool(name="cvp", bufs=2) as cvp, \
             tc.tile_pool(name="stage", bufs=4) as stg, \
             tc.tile_pool(name="ps", bufs=2, space="PSUM") as ps, \
             tc.tile_pool(name="psb", bufs=2, space="PSUM") as psb, \
             tc.tile_pool(name="psy", bufs=4, space="PSUM") as psy:

            # warm up the collective mesh immediately: the first collective
            # pays ~45us of one-time mesh setup, so fire a tiny one first
            if not os.environ.get("MAMBA_NO_AR"):
                warm = wp.tile([1, 16], F16)
                nc.vector.memset(warm[:], 0.0)
                nc.sync.dma_start(ar_warm_in[:], warm[:])
                nc.gpsimd.collective_compute(
                    "AllReduce", OP.add,
                    replica_groups=[list(range(NC_))],
                    ins=[ar_warm_in[:]], outs=[ar_warm_out[:]])

            # ---- weights / constants ----
            wxz_sb = wp.tile([128, 8 * 512], F16)
            for k in range(8):
                nc.sync.dma_start(wxz_sb[:, 512 * k:512 * (k + 1)], wxz[k])
            xpw_sb = wp.tile([128, 2 * 96], F16)
            dtw_sb = wp.tile([RNK, DL], F16)
            nc.sync.dma_start(dtw_sb[:], dtw[:])
            wo_sb = wp.tile([128, 2 * D], F16)
            cw_sb = wp.tile([128, 2 * KC], F32)
            cbh_sb = wp.tile([128, 2], F32)
            db2_sb = wp.tile([128, 2], F32)
            av_sb = wp.tile([128, 2 * NST], F32)
            dgd_sb = wp.tile([128, 2 * 128], F16)
            for k in range(2):
                nc.sync.dma_start(xpw_sb[:, 96 * k:96 * (k + 1)], xpw[k])
                nc.sync.dma_start(wo_sb[:, D * k:D * (k + 1)], wo[k])
                nc.sync.dma_start(cw_sb[:, KC * k:KC * (k + 1)], cw[k])
                nc.sync.dma_start(cbh_sb[:, k:k + 1], cbh[k])
                nc.sync.dma_start(db2_sb[:, k:k + 1], db2[k])
                nc.sync.dma_start(av_sb[:, NST * k:NST * (k + 1)], av[k])
                nc.sync.dma_start(dgd_sb[:, 128 * k:128 * (k + 1)], dgd[k])
            cdg_sb = wp.tile([128, 2 * KC * 128], F16)
            for k in range(2 * KC):
                nc.sync.dma_start(cdg_sb[:, 128 * k:128 * (k + 1)], cdg[k])
            cdgv = cdg_sb.rearrange("p (k m) -> p k m", k=2 * KC)
            id_sb = wp.tile([128, 128], F32)
            nc.sync.dma_start(id_sb[:], idm[:])
            idb = wp.tile([128, 128], F16)
            nc.vector.tensor_copy(idb[:], id_sb[:])
            selBC = wp.tile([2, 256], F16)
            nc.sync.dma_start(selBC[:], selbc[:])
            selB = selBC[:, 0:128]
            selC = selBC[:, 128:256]

            # ---- persistent activations ----
            xpad = [dpool.tile([128, B * PADL], F16, name=f"xpad{i}") for i in range(2)]
            zs = [dpool.tile([128, T], F16, name=f"zs{i}") for i in range(2)]
            u16 = [dpool.tile([128, T], F16, name=f"u16_{i}") for i in range(2)]
            delta16 = [dpool.tile([128, T], F16, name=f"delta16_{i}") for i in range(2)]
            du = [dpool.tile([128, T], F16, name=f"du{i}") for i in range(2)]
            yg16 = [dpool.tile([128, T], F16, name=f"yg{i}") for i in range(2)]
            xdbl = [dpool.tile([RNK, L], F16, name=f"xdbl{i}") for i in range(2)]
            hkt = [[dpool.tile([128, 512], F16, name=f"hkt{p}_{k}")
                    for k in range(8)] for p in range(2)]
            # chunk-0 scan states, chained into chunk 1 (per dt, per n)
            hsave = [[dpool.tile([128, HC], F16, name=f"hs{d}_{n}")
                      for n in range(NST)] for d in range(2)]

            cwv = cw_sb.rearrange("p (k m) -> p k m", k=2)
            wxzv = wxz_sb.rearrange("p (k m) -> p k m", k=8)
            wov = wo_sb.rearrange("p (k m) -> p k m", k=2)

            # zero the conv pads
            for dt_ in range(2):
                xp3 = xpad[dt_].rearrange("p (s l) -> p s l", s=B)
                nc.vector.memset(xp3[:, :, 0:KC - 1], 0.0)

            # ---- phase 1: in_proj, one 512-token block, 2 PSUM banks ----
            def in_proj_tb(tb, pair=None):
                if pair is None:
                    in_proj_tb(tb, pair=0)
                    in_proj_tb(tb, pair=1)
                    return
                hk = hkt[tb % 2]
                if pair == 0:
                    for k in range(8):
                        nc.sync.dma_start(
                            hk[k][:],
                            hsT[128 * k:128 * (k + 1), 512 * tb:512 * (tb + 1)])
                # x halves then z halves so only 2 banks are live at a time
                for pair in [pair]:       # 0: x0,x1   1: z0,z1
                    psx = [ps.tile([128, 512], F32, name=f"psx{pair}{i}", tag="ps")
                           for i in range(2)]
                    for k in range(8):
                        for i in range(2):
                            half = 2 * pair + i
                            nc.tensor.matmul(
                                psx[i][:],
                                wxzv[:, k, 128 * half:128 * (half + 1)],
                                hk[k][:],
                                start=(k == 0), stop=(k == 7))
                    for i in range(2):
                        dt_ = i
                        if pair == 0:   # x -> padded conv layout (f16)
                            b_ = tb // 2
                            off = b_ * PADL + (KC - 1) + 512 * (tb % 2)
                            nc.scalar.copy(xpad[dt_][:, off:off + 512], psx[i][:])
                        else:           # z -> silu(z) = (tanh(z/2)+1) * (z/2)
                            sl5 = slice(512 * tb, 512 * (tb + 1))
                            sg = stream.tile([128, 512], F16, name="sg", tag="sg",
                                             bufs=2)
                            nc.scalar.activation(sg[:], psx[i][:], AF.Tanh, scale=0.5)
                            nc.scalar.activation(zs[dt_][:, sl5], psx[i][:],
                                                 AF.Identity, scale=0.5)
                            nc.vector.scalar_tensor_tensor(
                                zs[dt_][:, sl5], sg[:], 1.0, zs[dt_][:, sl5],
                                op0=OP.add, op1=OP.mult)

            # ---- phase 2: depthwise causal conv + silu -> u16, per chunk ----
            def conv_bc(b_, c_):
                # depthwise conv as 4 diag-matmul taps accumulating in PSUM
                tsl = slice(L * b_ + HC * c_, L * b_ + HC * (c_ + 1))
                x0 = b_ * PADL + HC * c_   # xpad col of (token - (KC-1))
                for dt_ in range(2):
                    pc = ps.tile([128, HC], F32, name="pc", tag="ps")
                    for k in range(KC):
                        nc.tensor.matmul(
                            pc[:], cdgv[:, KC * dt_ + k, :],
                            xpad[dt_][:, x0 + k:x0 + k + HC],
                            start=(k == 0), stop=(k == KC - 1))
                    sgt = cvp.tile([128, HC], F16, name="sgt", tag="cv0")
                    # u = (acc+cb)*sigmoid(acc+cb) = (tanh(acc/2+cb/2)+1)*(acc/2+cb/2)
                    nc.scalar.activation(sgt[:], pc[:], AF.Tanh, scale=0.5,
                                         bias=cbh_sb[:, dt_:dt_ + 1])
                    nc.scalar.activation(u16[dt_][:, tsl], pc[:], AF.Identity,
                                         scale=0.5, bias=cbh_sb[:, dt_:dt_ + 1])
                    nc.vector.scalar_tensor_tensor(
                        u16[dt_][:, tsl], sgt[:], 1.0, u16[dt_][:, tsl],
                        op0=OP.add, op1=OP.mult)

            # ---- phase 3: x_proj partial -> per-(batch,chunk) AllReduce#1 ----
            def xproj_ar1(b_, c_):
                tb = 2 * b_ + c_
                ps96 = ps.tile([96, 512], F32, name="ps96", tag="ps")
                for k in range(2):
                    nc.tensor.matmul(
                        ps96[:], xpw_sb[:, 96 * k:96 * (k + 1)],
                        u16[k][:, 512 * tb:512 * (tb + 1)],
                        start=(k == 0), stop=(k == 1))
                st = stg.tile([96, 512], F16, name="st_xp", tag="xp")
                nc.scalar.copy(st[:], ps96[:])
                nc.sync.dma_start(ar1_in[b_][c_][:], st[:])
                if os.environ.get("MAMBA_NO_AR"):
                    nc.sync.dma_start(ar1_out[b_][c_][:], ar1_in[b_][c_][:])
                else:
                    nc.gpsimd.collective_compute(
                        "AllReduce", OP.add,
                        replica_groups=[list(range(NC_))],
                        ins=[ar1_in[b_][c_][:]], outs=[ar1_out[b_][c_][:]])
                nc.sync.dma_start(xdbl[b_][:, HC * c_:HC * (c_ + 1)],
                                  ar1_out[b_][c_][0:RNK, :])

            # ---- per (batch, chunk): dt_proj + softplus + du ----
            def emit_delta(b_, c_):
                sl5 = slice(L * b_ + HC * c_, L * b_ + HC * (c_ + 1))
                for dt_ in range(2):
                    psd = ps.tile([128, 512], F32, name="psd", tag="ps")
                    nc.tensor.matmul(
                        psd[:], dtw_sb[:, 128 * dt_:128 * (dt_ + 1)],
                        xdbl[b_][:, HC * c_:HC * (c_ + 1)],
                        start=True, stop=True)
                    nc.scalar.activation(delta16[dt_][:, sl5], psd[:],
                                         AF.Exp, bias=db2_sb[:, dt_:dt_ + 1])
                    nc.scalar.activation(delta16[dt_][:, sl5], delta16[dt_][:, sl5],
                                         AF.Ln, bias=1.0)
                    nc.vector.tensor_mul(du[dt_][:, sl5], delta16[dt_][:, sl5],
                                         u16[dt_][:, sl5])

            # ---- scan: chunk-major; h chained c0 -> c1 via hsave ----
            def scan_chunk(b_, c_, inject):
                tsl = slice(L * b_ + HC * c_, L * b_ + HC * (c_ + 1))
                py = [psy.tile([128, 512], F32, name=f"py{b_}{c_}_{i}", tag="psy")
                      for i in range(2)]
                ar1v = ar1_out[b_][c_].rearrange("(g r) t -> g r t", r=NST)
                bcn = {}

                def fetch_bc(n):
                    t = bcp.tile([2, HC], F16, name=f"bcn{n}", tag="bcn", bufs=4)
                    nc.sync.dma_start(t[0:2, :], ar1v[4:6, n, :])
                    bcn[n] = t

                fetch_bc(0)
                fetch_bc(1)
                for n in range(NST):
                    if n in inject:
                        inject[n]()
                    if n + 2 < NST:
                        fetch_bc(n + 2)
                    bbc = bcp.tile([128, HC], F16, name="bbc", tag="bbc")
                    cbc = bcp.tile([128, HC], F16, name="cbc", tag="cbc")
                    pB = psb.tile([128, 512], F32, name="pB", tag="psb")
                    nc.tensor.matmul(pB[:], selB[:], bcn[n][:], start=True,
                                     stop=True)
                    nc.scalar.copy(bbc[:], pB[:])
                    pC = psb.tile([128, 512], F32, name="pC", tag="psb")
                    nc.tensor.matmul(pC[:], selC[:], bcn[n][:], start=True,
                                     stop=True)
                    nc.scalar.copy(cbc[:], pC[:])
                    for dt_ in range(2):
                        dA = scp.tile([128, HC], F32, name="dA", tag=f"dA{dt_}",
                                      bufs=2)
                        nc.scalar.activation(
                            dA[:], delta16[dt_][:, tsl], AF.Exp,
                            scale=av_sb[:, NST * dt_ + n:NST * dt_ + n + 1])
                        dbu = scp.tile([128, HC], F16, name="dbu",
                                       tag=f"dbu{dt_}", bufs=3)
                        nc.vector.tensor_mul(dbu[:], du[dt_][:, tsl], bbc[:])
                        if c_ == 0:
                            h = hsave[dt_][n]
                            nc.vector.tensor_tensor_scan(
                                h[:], dA[:], dbu[:], 0.0,
                                op0=OP.mult, op1=OP.add)
                        else:
                            h = scp.tile([128, HC], F16, name="h",
                                         tag=f"h{dt_}", bufs=2)
                            nc.vector.tensor_tensor_scan(
                                h[:], dA[:], dbu[:],
                                hsave[dt_][n][:, HC - 1:HC],
                                op0=OP.mult, op1=OP.add)
                        ch = scp.tile([128, HC], F16, name="ch",
                                      tag=f"ch{dt_}", bufs=2)
                        nc.vector.tensor_mul(ch[:], h[:], cbc[:])
                        nc.tensor.matmul(py[dt_][:], idb[:], ch[:],
                                         start=(n == 0), stop=False)
                return py

            def backend(b_, c_, py):
                # skip connection (closes psum), gate, out_proj, RS
                for dt_ in range(2):
                    nc.tensor.matmul(
                        py[dt_][:],
                        dgd_sb[:, 128 * dt_:128 * (dt_ + 1)],
                        u16[dt_][:, L * b_ + HC * c_:L * b_ + HC * (c_ + 1)],
                        start=False, stop=True)
                for dt_ in range(2):
                    sl5 = slice(L * b_ + HC * c_, L * b_ + HC * (c_ + 1))
                    nc.vector.tensor_mul(yg16[dt_][:, sl5], py[dt_][:],
                                         zs[dt_][:, sl5])
                # out_proj matmuls/stores, then one ReduceScatter per chunk
                for tt in range(4 * c_, 4 * (c_ + 1)):
                    t0 = L * b_ + 128 * tt
                    for mb in range(2):
                        pso = ps.tile([128, 512], F32, name="pso", tag="ps")
                        for k in range(2):
                            nc.tensor.matmul(
                                pso[:],
                                yg16[k][:, t0:t0 + 128],
                                wov[:, k, 512 * mb:512 * (mb + 1)],
                                start=(k == 0), stop=(k == 1))
                        st = stg.tile([128, 512], F16, name="st_op",
                                      tag="op", bufs=4)
                        nc.scalar.copy(st[:], pso[:])
                        nc.sync.dma_start(
                            ar2_in[b_][128 * tt:128 * (tt + 1),
                                       512 * mb:512 * (mb + 1)], st[:])
                hsl = slice(512 * c_, 512 * (c_ + 1))
                osl = slice(64 * c_, 64 * (c_ + 1))
                if os.environ.get("MAMBA_NO_AR"):
                    nc.sync.dma_start(ar2_out[b_][osl, :], ar2_in[b_][osl, :])
                else:
                    nc.gpsimd.collective_compute(
                        "ReduceScatter", OP.add,
                        replica_groups=[list(range(NC_))],
                        ins=[ar2_in[b_][hsl, :]], outs=[ar2_out[b_][osl, :]])
                nc.sync.dma_start(outp[b_][osl, :], ar2_out[b_][osl, :])

            # ---- emission ----
            in_proj_tb(0, pair=0)
            conv_bc(0, 0)
            xproj_ar1(0, 0)
            in_proj_tb(0, pair=1)
            in_proj_tb(1, pair=0)
            conv_bc(0, 1)
            in_proj_tb(1, pair=1)
            in_proj_tb(2)
            xproj_ar1(0, 1)
            emit_delta(0, 0)
            in_proj_tb(3)
            py00 = scan_chunk(0, 0, inject={
                2: lambda: conv_bc(1, 0),
                5: lambda: xproj_ar1(1, 0),
                8: lambda: emit_delta(0, 1),
                11: lambda: conv_bc(1, 1),
            })
            py01 = scan_chunk(0, 1, inject={
                1: lambda: backend(0, 0, py00),
                4: lambda: xproj_ar1(1, 1),
                8: lambda: emit_delta(1, 0),
            })
            py10 = scan_chunk(1, 0, inject={
                1: lambda: backend(0, 1, py01),
                6: lambda: emit_delta(1, 1),
            })
            py11 = scan_chunk(1, 1, inject={
                1: lambda: backend(1, 0, py10),
            })
            backend(1, 1, py11)
    nc.finalize()
    return nc


def make_in_maps(inputs):
    hs = np.asarray(inputs["hidden_states"], np.float32)
    ipw = np.asarray(inputs["in_proj_w"], np.float32)
    cw = np.asarray(inputs["conv_w"], np.float32)
    cb = np.asarray(inputs["conv_b"], np.float32)
    xpw = np.asarray(inputs["x_proj_w"], np.float32)
    dtw = np.asarray(inputs["dt_proj_w"], np.float32)
    dtb = np.asarray(inputs["dt_proj_b"], np.float32)
    alog = np.asarray(inputs["A_log"], np.float32)
    dvec = np.asarray(inputs["D"], np.float32)
    wo = np.asarray(inputs["out_proj_w"], np.float32)

    hsT = np.ascontiguousarray(hs.transpose(2, 0, 1).reshape(D, T)).astype(np.float16)
    ident = np.eye(128, dtype=np.float32)
    selbc_np = np.zeros((2, 256), np.float16)
    selbc_np[0, 0:128] = 1.0   # selB row 0
    selbc_np[1, 128:256] = 1.0  # selC row 1

    in_maps = []
    for c in range(NC_):
        sl = slice(DL * c, DL * (c + 1))
        wxzT = np.concatenate([ipw[sl].T, ipw[DI + DL * c: DI + DL * (c + 1)].T],
                              axis=1)                      # [1024, 512]
        dloc = dvec[sl]
        dgd = np.stack([np.diag(dloc[0:128]), np.diag(dloc[128:256])])
        m = {
            "hsT": hsT,
            "wxzT": np.ascontiguousarray(wxzT.reshape(8, 128, 512)).astype(np.float16),
            "xpwT": np.ascontiguousarray(xpw[:, sl].T.reshape(2, 128, 96)).astype(np.float16),
            "dtwT": np.ascontiguousarray(dtw[sl].T).astype(np.float16),
            "woT": np.ascontiguousarray(wo[:, sl].T.reshape(2, 128, D)).astype(np.float16),
            "convw": np.ascontiguousarray(cw[sl, 0, :].reshape(2, 128, KC)),
            "convdg": np.ascontiguousarray(np.stack(
                [np.diag(cw[sl, 0, :].reshape(2, 128, KC)[dt_, :, k])
                 for dt_ in range(2) for k in range(KC)])).astype(np.float16),
            "convbh": np.ascontiguousarray((0.5 * cb[sl]).reshape(2, 128, 1)),
            "dtb2p": np.ascontiguousarray((2.0 * dtb[sl]).reshape(2, 128, 1)),
            "Aneg": np.ascontiguousarray((-np.exp(alog[sl])).reshape(2, 128, NST)),
            "diagD": np.ascontiguousarray(dgd).astype(np.float16),
            "ident": ident,
            "selbc": selbc_np,
        }
        in_maps.append(m)
    return in_maps


def assemble_output(results):
    # RS per 512-token chunk c: core k receives tokens [512c + 64k, +64),
    # stored at rows [64c, +64).
    out = np.zeros((B, L, D), np.float32)
    for k in range(NC_):
        s = np.asarray(results[k]["out"], np.float32)  # [B, 128, D]
        for b_ in range(B):
            for c in range(2):
                out[b_, 512 * c + 64 * k: 512 * c + 64 * (k + 1), :] = \
                    s[b_][64 * c:64 * (c + 1)]
    return out


def kernel(**inputs):
    from concourse.bass_utils import run_bass_kernel_spmd
    nc = build_nc()
    in_maps = make_in_maps(inputs)
    trace = bool(int(os.environ.get("MAMBA_TRACE", "0")))
    tdir = os.environ.get("MAMBA_TRACE_DIR") if trace else None
    if tdir:
        os.makedirs(tdir, exist_ok=True)
    res = run_bass_kernel_spmd(nc, in_maps, list(range(NC_)), trace=trace,
                               tmpdir=tdir)
    if trace and res.exec_time_ns is not None:
        print(f"HW exec time: {res.exec_time_ns} ns")
        if res.instructions_and_trace is not None:
            print(f"trace path: {res.instructions_and_trace[1]}")
    return assemble_output(res.results)


# revision 41
# speedup vs baseline: 1.2072x; 1.0165x over previous
"""Mamba block Trainium2 kernel, 8-way tensor-parallel over d_inner.

Shapes (hardcoded from the problem spec):
  hidden_states [2, 1024, 1024], d_model=1024, d_inner=2048, d_state=16,
  dt_rank=64, d_conv=4.  Each core owns DL=256 d_inner channels.

v4: chunk-major scan (512-token chunks, h chained via persistent per-state
tiles) so each chunk's out_proj+ReduceScatter overlaps the next chunk's
scan loop and the first scan starts ~50us in.  B/C broadcast via PE
selector matmuls -> PSUM -> Act copies -> SBUF f16 (no DMA broadcast).
All scan-phase elementwise work on DVE (@512 f16 fast path) + Act; the
Pool engine is left idle on purpose: concurrent GpSimd ops halve DVE
throughput (shared SBUF ports).  AR1 per (batch, chunk).
"""
import sys, os
sys.path.insert(0, "/opt/trn_rl_repo")
import numpy as np

import concourse.bass as bass
import concourse.bacc as bacc
import concourse.mybir as mybir
import concourse.tile as tile

F32 = mybir.dt.float32
F16 = mybir.dt.float16
BF16 = mybir.dt.bfloat16
AF = mybir.ActivationFunctionType
OP = mybir.AluOpType

B, L, D, DI, NST, RNK, KC = 2, 1024, 1024, 2048, 16, 64, 4
NC_ = 8
DL = DI // NC_          # 256 local channels
T = B * L               # 2048 tokens
HC = 512                # chunk length (tokens)


def build_nc():
    nc = bacc.Bacc()
    dp = nc.declare_dram_parameter
    hsT = dp("hsT", [D, T], F16, isOutput=False)             # hidden^T fp16
    wxz = dp("wxzT", [8, 128, 512], F16, isOutput=False)     # in_proj^T k-tiles
    xpw = dp("xpwT", [2, 128, 96], F16, isOutput=False)      # x_proj^T k-tiles
    dtw = dp("dtwT", [RNK, DL], F16, isOutput=False)         # dt_proj^T
    wo = dp("woT", [2, 128, D], F16, isOutput=False)         # out_proj^T k-tiles
    cw = dp("convw", [2, 128, KC], F32, isOutput=False)
    cdg = dp("convdg", [2 * KC, 128, 128], F16, isOutput=False)  # diag taps
    cbh = dp("convbh", [2, 128, 1], F32, isOutput=False)     # conv_b / 2
    db2 = dp("dtb2p", [2, 128, 1], F32, isOutput=False)      # +2*dt_proj_b
    av = dp("Aneg", [2, 128, NST], F32, isOutput=False)      # -exp(A_log)
    dgd = dp("diagD", [2, 128, 128], F16, isOutput=False)    # diag(D) per d-tile
    idm = dp("ident", [128, 128], F32, isOutput=False)
    selbc = dp("selbc", [2, 256], F16, isOutput=False)
    outp = dp("out", [B, 128, D], F16, isOutput=True)        # per-core RS slice

    ar1_in = [[nc.dram_tensor(f"ar1_in{b}_{c}", [96, HC], F16)
               for c in range(2)] for b in range(B)]
    ar1_out = [[nc.dram_tensor(f"ar1_out{b}_{c}", [96, HC], F16,
                               addr_space="Shared")
                for c in range(2)] for b in range(B)]
    ar2_in = [nc.dram_tensor(f"ar2_in{b}", [L, D], F16) for b in range(B)]
    ar2_out = [nc.dram_tensor(f"ar2_out{b}", [128, D], F16) for b in range(B)]
    ar_warm_in = nc.dram_tensor("ar_warm_in", [1, 16], F16)
    ar_warm_out = nc.dram_tensor("ar_warm_out", [1, 16], F16, addr_space="Shared")

    PADL = L + KC - 1  # 1027 per batch segment

    with tile.TileContext(nc) as tc:
        with tc.tile_pool(name="wp", bufs=1) as wp, \
             tc.tile_pool(name="data", bufs=1) as dpool, \
             tc.tile_pool(name="stream", bufs=6) as stream, \
             tc.tile_pool(name="scan", bufs=3) as scp, \
             tc.tile_pool(name="bcp", bufs=3) as bcp, \
             tc.tile_pool(name="cvp", bufs=2) as cvp, \
             tc.tile_pool(name="stage", bufs=4) as stg, \
             tc.tile_pool(name="ps", bufs=2, space="PSUM") as ps, \
             tc.tile_pool(name="psb", bufs=2, space="PSUM") as psb, \
             tc.tile_pool(name="psy", bufs=4, space="PSUM") as psy:

            # warm up the collective mesh immediately: the first collective
            # pays ~45us of one-time mesh setup, so fire a tiny one first
            if not os.environ.get("MAMBA_NO_AR"):
                warm = wp.tile([1, 16], F16)
                nc.vector.memset(warm[:], 0.0)
                nc.sync.dma_start(ar_warm_in[:], warm[:])
                nc.gpsimd.collective_compute(
                    "AllReduce", OP.add,
                    replica_groups=[list(range(NC_))],
                    ins=[ar_warm_in[:]], outs=[ar_warm_out[:]])

            # ---- weights / constants ----
            wxz_sb = wp.tile([128, 8 * 512], F16)
            for k in range(8):
                nc.sync.dma_start(wxz_sb[:, 512 * k:512 * (k + 1)], wxz[k])
            xpw_sb = wp.tile([128, 2 * 96], F16)
            dtw_sb = wp.tile([RNK, DL], F16)
            nc.sync.dma_start(dtw_sb[:], dtw[:])
            wo_sb = wp.tile([128, 2 * D], F16)
            cw_sb = wp.tile([128, 2 * KC], F32)
            cbh_sb = wp.tile([128, 2], F32)
            db2_sb = wp.tile([128, 2], F32)
            av_sb = wp.tile([128, 2 * NST], F32)
            dgd_sb = wp.tile([128, 2 * 128], F16)
            for k in range(2):
                nc.sync.dma_start(xpw_sb[:, 96 * k:96 * (k + 1)], xpw[k])
                nc.sync.dma_start(wo_sb[:, D * k:D * (k + 1)], wo[k])
                nc.sync.dma_start(cw_sb[:, KC * k:KC * (k + 1)], cw[k])
                nc.sync.dma_start(cbh_sb[:, k:k + 1], cbh[k])
                nc.sync.dma_start(db2_sb[:, k:k + 1], db2[k])
                nc.sync.dma_start(av_sb[:, NST * k:NST * (k + 1)], av[k])
                nc.sync.dma_start(dgd_sb[:, 128 * k:128 * (k + 1)], dgd[k])
            cdg_sb = wp.tile([128, 2 * KC * 128], F16)
            for k in range(2 * KC):
                nc.sync.dma_start(cdg_sb[:, 128 * k:128 * (k + 1)], cdg[k])
            cdgv = cdg_sb.rearrange("p (k m) -> p k m", k=2 * KC)
            id_sb = wp.tile([128, 128], F32)
            nc.sync.dma_start(id_sb[:], idm[:])
            idb = wp.tile([128, 128], F16)
            nc.vector.tensor_copy(idb[:], id_sb[:])
            selBC = wp.tile([2, 256], F16)
            nc.sync.dma_start(selBC[:], selbc[:])
            selB = selBC[:, 0:128]
            selC = selBC[:, 128:256]

            # ---- persistent activations ----
            xpad = [dpool.tile([128, B * PADL], F16, name=f"xpad{i}") for i in range(2)]
            zs = [dpool.tile([128, T], F16, name=f"zs{i}") for i in range(2)]
            u16 = [dpool.tile([128, T], F16, name=f"u16_{i}") for i in range(2)]
            delta16 = [dpool.tile([128, T], F16, name=f"delta16_{i}") for i in range(2)]
            du = [dpool.tile([128, T], F16, name=f"du{i}") for i in range(2)]
            yg16 = [dpool.tile([128, T], F16, name=f"yg{i}") for i in range(2)]
            xdbl = [dpool.tile([RNK, L], F16, name=f"xdbl{i}") for i in range(2)]
            hkt = [[dpool.tile([128, 512], F16, name=f"hkt{p}_{k}")
                    for k in range(8)] for p in range(2)]
            # chunk-0 scan states, chained into chunk 1 (per dt, per n)
            hsave = [[dpool.tile([128, HC], F16, name=f"hs{d}_{n}")
                      for n in range(NST)] for d in range(2)]

            cwv = cw_sb.rearrange("p (k m) -> p k m", k=2)
            wxzv = wxz_sb.rearrange("p (k m) -> p k m", k=8)
            wov = wo_sb.rearrange("p (k m) -> p k m", k=2)

            # zero the conv pads
            for dt_ in range(2):
                xp3 = xpad[dt_].rearrange("p (s l) -> p s l", s=B)
                nc.vector.memset(xp3[:, :, 0:KC - 1], 0.0)

            # ---- phase 1: in_proj, one 512-token block, 2 PSUM banks ----
            def in_proj_tb(tb, pair=None):
                if pair is None:
                    in_proj_tb(tb, pair=0)
                    in_proj_tb(tb, pair=1)
                    return
                hk = hkt[tb % 2]
                if pair == 0:
                    for k in range(8):
                        nc.sync.dma_start(
                            hk[k][:],
                            hsT[128 * k:128 * (k + 1), 512 * tb:512 * (tb + 1)])
                # x halves then z halves so only 2 banks are live at a time
                for pair in [pair]:       # 0: x0,x1   1: z0,z1
                    psx = [ps.tile([128, 512], F32, name=f"psx{pair}{i}", tag="ps")
                           for i in range(2)]
                    for k in range(8):
                        for i in range(2):
                            half = 2 * pair + i
                            nc.tensor.matmul(
                                psx[i][:],
                                wxzv[:, k, 128 * half:128 * (half + 1)],
                                hk[k][:],
                                start=(k == 0), stop=(k == 7))
                    for i in range(2):
                        dt_ = i
                        if pair == 0:   # x -> padded conv layout (f16)
                            b_ = tb // 2
                            off = b_ * PADL + (KC - 1) + 512 * (tb % 2)
                            nc.scalar.copy(xpad[dt_][:, off:off + 512], psx[i][:])
                        else:           # z -> silu(z) = (tanh(z/2)+1) * (z/2)
                            sl5 = slice(512 * tb, 512 * (tb + 1))
                            sg = stream.tile([128, 512], F16, name="sg", tag="sg",
                                             bufs=2)
                            nc.scalar.activation(sg[:], psx[i][:], AF.Tanh, scale=0.5)
                            nc.scalar.activation(zs[dt_][:, sl5], psx[i][:],
                                                 AF.Identity, scale=0.5)
                            nc.vector.scalar_tensor_tensor(
                                zs[dt_][:, sl5], sg[:], 1.0, zs[dt_][:, sl5],
                                op0=OP.add, op1=OP.mult)

            # ---- phase 2: depthwise causal conv + silu -> u16, per chunk ----
            def conv_bc(b_, c_):
                # depthwise conv as 4 diag-matmul taps accumulating in PSUM
                tsl = slice(L * b_ + HC * c_, L * b_ + HC * (c_ + 1))
                x0 = b_ * PADL + HC * c_   # xpad col of (token - (KC-1))
                for dt_ in range(2):
                    pc = ps.tile([128, HC], F32, name="pc", tag="ps")
                    for k in range(KC):
                        nc.tensor.matmul(
                            pc[:], cdgv[:, KC * dt_ + k, :],
                            xpad[dt_][:, x0 + k:x0 + k + HC],
                            start=(k == 0), stop=(k == KC - 1))
                    sgt = cvp.tile([128, HC], F16, name="sgt", tag="cv0")
                    # u = (acc+cb)*sigmoid(acc+cb) = (tanh(acc/2+cb/2)+1)*(acc/2+cb/2)
                    nc.scalar.activation(sgt[:], pc[:], AF.Tanh, scale=0.5,
                                         bias=cbh_sb[:, dt_:dt_ + 1])
                    nc.scalar.activation(u16[dt_][:, tsl], pc[:], AF.Identity,
                                         scale=0.5, bias=cbh_sb[:, dt_:dt_ + 1])
                    nc.vector.scalar_tensor_tensor(
                        u16[dt_][:, tsl], sgt[:], 1.0, u16[dt_][:, tsl],
                        op0=OP.add, op1=OP.mult)

            # ---- phase 3: x_proj partial -> per-(batch,chunk) AllReduce#1 ----
            def xproj_ar1(b_, c_):
                tb = 2 * b_ + c_
                ps96 = ps.tile([96, 512], F32, name="ps96", tag="ps")
                for k in range(2):
                    nc.tensor.matmul(
                        ps96[:], xpw_sb[:, 96 * k:96 * (k + 1)],
                        u16[k][:, 512 * tb:512 * (tb + 1)],
                        start=(k == 0), stop=(k == 1))
                st = stg.tile([96, 512], F16, name="st_xp", tag="xp")
                nc.scalar.copy(st[:], ps96[:])
                nc.sync.dma_start(ar1_in[b_][c_][:], st[:])
                if os.environ.get("MAMBA_NO_AR"):
                    nc.sync.dma_start(ar1_out[b_][c_][:], ar1_in[b_][c_][:])
                else:
                    nc.gpsimd.collective_compute(
                        "AllReduce", OP.add,
                        replica_groups=[list(range(NC_))],
                        ins=[ar1_in[b_][c_][:]], outs=[ar1_out[b_][c_][:]])
                nc.sync.dma_start(xdbl[b_][:, HC * c_:HC * (c_ + 1)],
                                  ar1_out[b_][c_][0:RNK, :])

            # ---- per (batch, chunk): dt_proj + softplus + du ----
            def emit_delta(b_, c_):
                sl5 = slice(L * b_ + HC * c_, L * b_ + HC * (c_ + 1))
                for dt_ in range(2):
                    psd = ps.tile([128, 512], F32, name="psd", tag="ps")
                    nc.tensor.matmul(
                        psd[:], dtw_sb[:, 128 * dt_:128 * (dt_ + 1)],
                        xdbl[b_][:, HC * c_:HC * (c_ + 1)],
                        start=True, stop=True)
                    nc.scalar.activation(delta16[dt_][:, sl5], psd[:],
                                         AF.Exp, bias=db2_sb[:, dt_:dt_ + 1])
                    nc.scalar.activation(delta16[dt_][:, sl5], delta16[dt_][:, sl5],
                                         AF.Ln, bias=1.0)
                    nc.vector.tensor_mul(du[dt_][:, sl5], delta16[dt_][:, sl5],
                                         u16[dt_][:, sl5])

            # ---- scan: chunk-major; h chained c0 -> c1 via hsave ----
            def scan_chunk(b_, c_, inject):
                tsl = slice(L * b_ + HC * c_, L * b_ + HC * (c_ + 1))
                py = [psy.tile([128, 512], F32, name=f"py{b_}{c_}_{i}", tag="psy")
                      for i in range(2)]
                ar1v = ar1_out[b_][c_].rearrange("(g r) t -> g r t", r=NST)
                bcn = {}

                def fetch_bc(n):
                    t = bcp.tile([2, HC], F16, name=f"bcn{n}", tag="bcn", bufs=6)
                    nc.sync.dma_start(t[0:2, :], ar1v[4:6, n, :])
                    bcn[n] = t

                fetch_bc(0)
                fetch_bc(1)
                for n in range(NST):
                    if n in inject:
                        inject[n]()
                    if n + 2 < NST:
                        fetch_bc(n + 2)
                    bbc = bcp.tile([128, HC], F16, name="bbc", tag="bbc", bufs=4)
                    cbc = bcp.tile([128, HC], F16, name="cbc", tag="cbc", bufs=4)
                    pB = psb.tile([128, 512], F32, name="pB", tag="psb")
                    nc.tensor.matmul(pB[:], selB[:], bcn[n][:], start=True,
                                     stop=True)
                    nc.scalar.copy(bbc[:], pB[:])
                    pC = psb.tile([128, 512], F32, name="pC", tag="psb")
                    nc.tensor.matmul(pC[:], selC[:], bcn[n][:], start=True,
                                     stop=True)
                    nc.scalar.copy(cbc[:], pC[:])
                    for dt_ in range(2):
                        dA = scp.tile([128, HC], F32, name="dA", tag=f"dA{dt_}",
                                      bufs=3)
                        nc.scalar.activation(
                            dA[:], delta16[dt_][:, tsl], AF.Exp,
                            scale=av_sb[:, NST * dt_ + n:NST * dt_ + n + 1])
                        dbu = scp.tile([128, HC], F16, name="dbu",
                                       tag=f"dbu{dt_}", bufs=4)
                        nc.vector.tensor_mul(dbu[:], du[dt_][:, tsl], bbc[:])
                        if c_ == 0:
                            h = hsave[dt_][n]
                            nc.vector.tensor_tensor_scan(
                                h[:], dA[:], dbu[:], 0.0,
                                op0=OP.mult, op1=OP.add)
                        else:
                            h = scp.tile([128, HC], F16, name="h",
                                         tag=f"h{dt_}", bufs=2)
                            nc.vector.tensor_tensor_scan(
                                h[:], dA[:], dbu[:],
                                hsave[dt_][n][:, HC - 1:HC],
                                op0=OP.mult, op1=OP.add)
                        ch = scp.tile([128, HC], F16, name="ch",
                                      tag=f"ch{dt_}", bufs=2)
                        nc.vector.tensor_mul(ch[:], h[:], cbc[:])
                        nc.tensor.matmul(py[dt_][:], idb[:], ch[:],
                                         start=(n == 0), stop=False)
                return py

            def backend(b_, c_, py):
                # skip connection (closes psum), gate, out_proj, RS
                for dt_ in range(2):
                    nc.tensor.matmul(
                        py[dt_][:],
                        dgd_sb[:, 128 * dt_:128 * (dt_ + 1)],
                        u16[dt_][:, L * b_ + HC * c_:L * b_ + HC * (c_ + 1)],
                        start=False, stop=True)
                for dt_ in range(2):
                    sl5 = slice(L * b_ + HC * c_, L * b_ + HC * (c_ + 1))
                    nc.vector.tensor_mul(yg16[dt_][:, sl5], py[dt_][:],
                                         zs[dt_][:, sl5])
                # out_proj matmuls/stores, then one ReduceScatter per chunk
                for tt in range(4 * c_, 4 * (c_ + 1)):
                    t0 = L * b_ + 128 * tt
                    for mb in range(2):
                        pso = ps.tile([128, 512], F32, name="pso", tag="ps")
                        for k in range(2):
                            nc.tensor.matmul(
                                pso[:],
                                yg16[k][:, t0:t0 + 128],
                                wov[:, k, 512 * mb:512 * (mb + 1)],
                                start=(k == 0), stop=(k == 1))
                        st = stg.tile([128, 512], F16, name="st_op",
                                      tag="op", bufs=4)
                        nc.scalar.copy(st[:], pso[:])
                        nc.sync.dma_start(
                            ar2_in[b_][128 * tt:128 * (tt + 1),
                                       512 * mb:512 * (mb + 1)], st[:])
                hsl = slice(512 * c_, 512 * (c_ + 1))
                osl = slice(64 * c_, 64 * (c_ + 1))
                if os.environ.get("MAMBA_NO_AR"):
                    nc.sync.dma_start(ar2_out[b_][osl, :], ar2_in[b_][osl, :])
                else:
                    nc.gpsimd.collective_compute(
                        "ReduceScatter", OP.add,
                        replica_groups=[list(range(NC_))],
                        ins=[ar2_in[b_][hsl, :]], outs=[ar2_out[b_][osl, :]])
                nc.sync.dma_start(outp[b_][osl, :], ar2_out[b_][osl, :])

            # ---- emission ----
            in_proj_tb(0, pair=0)
            conv_bc(0, 0)
            xproj_ar1(0, 0)
            in_proj_tb(0, pair=1)
            in_proj_tb(1, pair=0)
            conv_bc(0, 1)
            in_proj_tb(1, pair=1)
            in_proj_tb(2)
            xproj_ar1(0, 1)
            emit_delta(0, 0)
            in_proj_tb(3)
            py00 = scan_chunk(0, 0, inject={
                2: lambda: conv_bc(1, 0),
                5: lambda: xproj_ar1(1, 0),
                8: lambda: emit_delta(0, 1),
                11: lambda: conv_bc(1, 1),
            })
            py01 = scan_chunk(0, 1, inject={
                2: lambda: backend(0, 0, py00),
                4: lambda: xproj_ar1(1, 1),
                8: lambda: emit_delta(1, 0),
            })
            py10 = scan_chunk(1, 0, inject={
                2: lambda: backend(0, 1, py01),
                6: lambda: emit_delta(1, 1),
            })
            py11 = scan_chunk(1, 1, inject={
                2: lambda: backend(1, 0, py10),
            })
            backend(1, 1, py11)
    nc.finalize()
    return nc


def make_in_maps(inputs):
    hs = np.asarray(inputs["hidden_states"], np.float32)
    ipw = np.asarray(inputs["in_proj_w"], np.float32)
    cw = np.asarray(inputs["conv_w"], np.float32)
    cb = np.asarray(inputs["conv_b"], np.float32)
    xpw = np.asarray(inputs["x_proj_w"], np.float32)
    dtw = np.asarray(inputs["dt_proj_w"], np.float32)
    dtb = np.asarray(inputs["dt_proj_b"], np.float32)
    alog = np.asarray(inputs["A_log"], np.float32)
    dvec = np.asarray(inputs["D"], np.float32)
    wo = np.asarray(inputs["out_proj_w"], np.float32)

    hsT = np.ascontiguousarray(hs.transpose(2, 0, 1).reshape(D, T)).astype(np.float16)
    ident = np.eye(128, dtype=np.float32)
    selbc_np = np.zeros((2, 256), np.float16)
    selbc_np[0, 0:128] = 1.0   # selB row 0
    selbc_np[1, 128:256] = 1.0  # selC row 1

    in_maps = []
    for c in range(NC_):
        sl = slice(DL * c, DL * (c + 1))
        wxzT = np.concatenate([ipw[sl].T, ipw[DI + DL * c: DI + DL * (c + 1)].T],
                              axis=1)                      # [1024, 512]
        dloc = dvec[sl]
        dgd = np.stack([np.diag(dloc[0:128]), np.diag(dloc[128:256])])
        m = {
            "hsT": hsT,
            "wxzT": np.ascontiguousarray(wxzT.reshape(8, 128, 512)).astype(np.float16),
            "xpwT": np.ascontiguousarray(xpw[:, sl].T.reshape(2, 128, 96)).astype(np.float16),
            "dtwT": np.ascontiguousarray(dtw[sl].T).astype(np.float16),
            "woT": np.ascontiguousarray(wo[:, sl].T.reshape(2, 128, D)).astype(np.float16),
            "convw": np.ascontiguousarray(cw[sl, 0, :].reshape(2, 128, KC)),
            "convdg": np.ascontiguousarray(np.stack(
                [np.diag(cw[sl, 0, :].reshape(2, 128, KC)[dt_, :, k])
                 for dt_ in range(2) for k in range(KC)])).astype(np.float16),
            "convbh": np.ascontiguousarray((0.5 * cb[sl]).reshape(2, 128, 1)),
            "dtb2p": np.ascontiguousarray((2.0 * dtb[sl]).reshape(2, 128, 1)),
            "Aneg": np.ascontiguousarray((-np.exp(alog[sl])).reshape(2, 128, NST)),
            "diagD": np.ascontiguousarray(dgd).astype(np.float16),
            "ident": ident,
            "selbc": selbc_np,
        }
        in_maps.append(m)
    return in_maps


def assemble_output(results):
    # RS per 512-token chunk c: core k receives tokens [512c + 64k, +64),
    # stored at rows [64c, +64).
    out = np.zeros((B, L, D), np.float32)
    for k in range(NC_):
        s = np.asarray(results[k]["out"], np.float32)  # [B, 128, D]
        for b_ in range(B):
            for c in range(2):
                out[b_, 512 * c + 64 * k: 512 * c + 64 * (k + 1), :] = \
                    s[b_][64 * c:64 * (c + 1)]
    return out


def kernel(**inputs):
    from concourse.bass_utils import run_bass_kernel_spmd
    nc = build_nc()
    in_maps = make_in_maps(inputs)
    trace = bool(int(os.environ.get("MAMBA_TRACE", "0")))
    tdir = os.environ.get("MAMBA_TRACE_DIR") if trace else None
    if tdir:
        os.makedirs(tdir, exist_ok=True)
    res = run_bass_kernel_spmd(nc, in_maps, list(range(NC_)), trace=trace,
                               tmpdir=tdir)
    if trace and res.exec_time_ns is not None:
        print(f"HW exec time: {res.exec_time_ns} ns")
        if res.instructions_and_trace is not None:
            print(f"trace path: {res.instructions_and_trace[1]}")
    return assemble_output(res.results)
